# revision 8
# baseline (speedup 1.0000x reference)
"""Dense transformer block on 8 TRN2 NeuronCores.

Sharding: data-parallel over batch (4 pairs) x Megatron tensor-parallel 2-way
within each pair (QKV/proj split over heads, MLP fc/cproj split over the 4096
hidden dim), with a pairwise AllReduce after the attention projection and
after the MLP projection.

Device layout is feature-major ("transposed"): activations live as
[d_model, tokens] so every matmul contracts along the partition dim with
naturally-laid-out weights and no on-device transposes. The host feeds x
pre-transposed (tile-packed) and re-assembles the output.

Attention: scores are computed transposed (S^T[k_pos, q_pos]); softmax needs
no max-subtraction (scores are O(1) by construction); the attention-forcing
reweight (w[k] for k >= idx) folds into the exp as a per-partition ln(w)
bias; the softmax denominator rides the attn@V matmul as a 65th ones-column
of V. Causal masking: future kpos tiles are not computed, diagonal 128x128
blocks get a triangular mask post-exp, diagonal-region matmuls are
column-narrowed.

The emission order is software-pipelined with a 2-stage skew so the PE always
has independent work while the AllReduces and LN stat round-trips are in
flight:  s1 = qkv+attn+proj+AR1-start, s2 = LN1+fc+cproj+AR2-start,
s3 = LN2+store;  order: s1(0) s1(1) s2(0) s1(2) s2(1) s3(0) ...
"""

import numpy as np
import ml_dtypes

import concourse.bacc as bacc
import concourse.mybir as mybir
import concourse.tile as tile
from concourse.bass_utils import run_bass_kernel_spmd

F32 = mybir.dt.float32
BF16 = mybir.dt.bfloat16
AF = mybir.ActivationFunctionType
OP = mybir.AluOpType

B, S, D, H, HD, FF = 4, 2048, 1024, 16, 64, 4096
N_CORES = 8
PAIRS = [[0, 1], [2, 3], [4, 5], [6, 7]]
CH = 512                 # tokens per pipeline chunk
NCH = S // CH            # 4
DT = D // 128            # 8 d-tiles
KT = S // 128            # 16 kpos tiles
HPC = H // 2             # heads per core (TP-2)
EPS = 1e-5
BF = ml_dtypes.bfloat16


def _build(use_bqk, use_bv, use_projb, use_cprojb, use_g1b1, use_g2b2):
    nc = bacc.Bacc("TRN2", target_bir_lowering=False, debug=False,
                   enable_asserts=True, num_devices=N_CORES)

    # tile-packed inputs: leading dim indexes [128, X] tiles, each contiguous
    xq = nc.dram_tensor("xq", [NCH * DT, 128, 512], F32, kind="ExternalInput")
    wqk = nc.dram_tensor("wqk", [16, 128, 512], BF16, kind="ExternalInput")
    bqk = nc.dram_tensor("bqk", [1024], F32, kind="ExternalInput")
    wv = nc.dram_tensor("wv", [8, 128, 512], BF16, kind="ExternalInput")
    bv = nc.dram_tensor("bv", [512], BF16, kind="ExternalInput")
    wproj = nc.dram_tensor("wproj", [8, 128, 512], BF16, kind="ExternalInput")
    projb = nc.dram_tensor("projb", [D], F32, kind="ExternalInput")
    wfc = nc.dram_tensor("wfc", [32, 128, 512], BF16, kind="ExternalInput")
    fcb = nc.dram_tensor("fcb", [2048], F32, kind="ExternalInput")
    wcproj = nc.dram_tensor("wcproj", [64, 128, 256], BF16,
                            kind="ExternalInput")
    cprojb = nc.dram_tensor("cprojb", [D], F32, kind="ExternalInput")
    g1 = nc.dram_tensor("g1", [D], F32, kind="ExternalInput")
    b1 = nc.dram_tensor("b1", [D], F32, kind="ExternalInput")
    g2 = nc.dram_tensor("g2", [D], F32, kind="ExternalInput")
    b2 = nc.dram_tensor("b2", [D], F32, kind="ExternalInput")
    lna = nc.dram_tensor("lna", [S], F32, kind="ExternalInput")
    tri = nc.dram_tensor("tri", [128, 128], BF16, kind="ExternalInput")
    # tile-packed output: [chunk*DT + dtile, 128, 512]; host reassembles
    out = nc.dram_tensor("out", [NCH * DT, 128, 512], F32,
                         kind="ExternalOutput")

    from contextlib import ExitStack
    with tile.TileContext(nc) as tc, ExitStack() as ctx:
        def pool(name, bufs, space="SBUF"):
            return ctx.enter_context(
                tc.tile_pool(name=name, bufs=bufs, space=space))

        const = pool("const", 1)
        wqk_p = pool("wqk_p", 8)
        wv_p = pool("wv_p", 8)
        wproj_p = pool("wproj_p", 4)
        wfc_p = pool("wfc_p", 8)
        wcproj_p = pool("wcproj_p", 18)
        xstage = pool("xstage", 1)
        xTb_p = pool("xTb_p", 8)
        qTb_p = pool("qTb_p", 4)
        pP = pool("pP", 3)
        attnTb_p = pool("attnTb_p", 4)
        den_p = pool("den_p", 1)
        den1_p = pool("den1_p", 1)
        tmp64_p = pool("tmp64_p", 1)
        recip_p = pool("recip_p", 1)
        recipb_p = pool("recipb_p", 2)
        arin_p = pool("arin_p", 2)
        art_p = pool("art_p", 8)
        cast_p = pool("cast_p", 2)
        sq_p = pool("sq_p", 2)
        strow_p = pool("strow_p", 1)
        bcast_p = pool("bcast_p", 2)
        nf_p = pool("nf_p", 2)
        xf2_p = pool("xf2_p", 2)
        nf2_p = pool("nf2_p", 2)
        nTb_p = pool("nTb_p", 9)
        gT_p = pool("gT_p", 16)
        mT_p = pool("mT_p", 2)
        mar_p = pool("mar_p", 8)
        hT_p = pool("hT_p", 2)
        psS = pool("psS", 2, "PSUM")
        psA = pool("psA", 2, "PSUM")
        psM = pool("psM", 2, "PSUM")
        psT = pool("psT", 2, "PSUM")
        dram = pool("dram", 4, "DRAM")

        # ---- persistent state + constants ----
        kt_sb = const.tile([128, 4 * S], BF16, name="kt_sb")
        kt_v = kt_sb[:].rearrange("p (r q) -> p r q", q=S)
        v_sb = const.tile([128, KT * 520], BF16, name="v_sb")
        v_v = v_sb[:].rearrange("p (t e) -> p t e", e=520)

        tri_sb = const.tile([128, 128], BF16, name="tri_sb")
        nc.sync.dma_start(out=tri_sb[:], in_=tri[:])
        lna_sb = const.tile([128, KT], F32, name="lna_sb")
        nc.sync.dma_start(out=lna_sb[:],
                          in_=lna.rearrange("(t p) -> p t", p=128))
        ones_col_b = const.tile([128, 1], BF16, name="ones_col_b")
        nc.vector.memset(ones_col_b[:], 1.0)
        eps_sb = const.tile([1, 1], F32, name="eps_sb")
        nc.vector.memset(eps_sb[:], EPS)
        fcb_sb = const.tile([128, 16], F32, name="fcb_sb")
        nc.sync.dma_start(out=fcb_sb[:],
                          in_=fcb.rearrange("(i p) -> p i", p=128))

        def vec8(name, t):
            sb = const.tile([128, DT], F32, name=name)
            nc.sync.dma_start(out=sb[:],
                              in_=t.rearrange("(i p) -> p i", p=128))
            return sb

        bqk_sb = vec8("bqk_sb", bqk) if use_bqk else None
        projb_sb = vec8("projb_sb", projb) if use_projb else None
        cprojb_sb = vec8("cprojb_sb", cprojb) if use_cprojb else None
        g1_sb = vec8("g1_sb", g1) if use_g1b1 else None
        b1_sb = vec8("b1_sb", b1) if use_g1b1 else None
        g2_sb = vec8("g2_sb", g2) if use_g2b2 else None
        b2_sb = vec8("b2_sb", b2) if use_g2b2 else None
        if use_bv:
            ones_row_b = const.tile([1, 128], BF16, name="ones_row_b")
            nc.vector.memset(ones_row_b[:], 1.0)
            bv_sb = const.tile([1, 512], BF16, name="bv_sb")
            nc.sync.dma_start(out=bv_sb[:],
                              in_=bv.rearrange("(o q) -> o q", o=1))

        def layernorm(src_t, out_mk, g_sb, b_sb, use_gb):
            """src_t: 8 f32 [128,512] tiles, scratched in place with t*rstd.
            out_mk(i, urstd_b, g_sb, b_sb, use_gb) writes the output tile."""
            ps_sumA = psT.tile([1, 512], F32, tag="pst", name="ps_sumA")
            ps_sumB = psT.tile([1, 512], F32, tag="pst", name="ps_sumB")
            for i in range(DT):
                tb = cast_p.tile([128, 512], BF16, name="tb")
                nc.vector.tensor_copy(tb[:], src_t[i][:])
                nc.tensor.matmul(ps_sumA[:], ones_col_b[:], tb[:],
                                 start=(i == 0), stop=(i == DT - 1))
                sqt = sq_p.tile([128, 512], BF16, name="sqt")
                nc.scalar.activation(sqt[:], src_t[i][:], AF.Square)
                nc.tensor.matmul(ps_sumB[:], ones_col_b[:], sqt[:],
                                 start=(i == 0), stop=(i == DT - 1))
            st = strow_p.tile([1, 3 * 512], F32, tag="st", name="st")
            sA, sB, sC = st[:, 0:512], st[:, 512:1024], st[:, 1024:1536]
            nc.scalar.activation(sA, ps_sumA[:], AF.Copy, scale=1.0 / D)  # u
            nc.scalar.activation(sB, ps_sumB[:], AF.Identity,
                                 bias=eps_sb[:], scale=1.0 / D)   # msq+eps
            nc.scalar.activation(sC, sA, AF.Square)               # u^2
            nc.vector.tensor_sub(sB, sB, sC)                      # var
            nc.vector.reciprocal(sC, sB)                          # 1/var
            nc.scalar.activation(sB, sC, AF.Sqrt)                 # rstd
            nc.vector.tensor_mul(sA, sA, sB)                      # u*rstd
            srd = dram.tile([2, 512], F32, tag="strow_d", name="srd")
            nc.sync.dma_start(out=srd[0:1, :], in_=sB)
            nc.sync.dma_start(out=srd[1:2, :], in_=sA)
            rstd_b = bcast_p.tile([128, 512], F32, name="rstd_b")
            nc.sync.dma_start(out=rstd_b[:],
                              in_=srd[0:1, :].partition_broadcast(128))
            urstd_b = bcast_p.tile([128, 512], F32, name="urstd_b")
            nc.sync.dma_start(out=urstd_b[:],
                              in_=srd[1:2, :].partition_broadcast(128))
            for i in range(DT):
                nc.vector.tensor_mul(src_t[i][:], src_t[i][:], rstd_b[:])
                out_mk(i, urstd_b, g_sb, b_sb, use_gb)

        state = {}

        # ================= stage 1: qkv + attention + proj + AR1 ============
        def s1(c):
            tok = slice(CH * c, CH * (c + 1))
            xTb_t = []
            for i in range(DT):
                xs = xstage.tile([128, 512], F32, name="xs")
                nc.sync.dma_start(out=xs[:], in_=xq[DT * c + i])
                xb = xTb_p.tile([128, 512], BF16, name="xb")
                nc.vector.tensor_copy(xb[:], xs[:])
                xTb_t.append(xb)

            qTb_t = []
            for cc in range(2):
                wt = []
                for d in range(DT):
                    w = wqk_p.tile([128, 512], BF16, name="w_qk")
                    nc.sync.dma_start(out=w[:], in_=wqk[8 * cc + d])
                    wt.append(w)
                for ct in range(4):
                    i = 4 * cc + ct
                    ps = psM.tile([128, 512], F32, tag="mm", name="ps_qk")
                    for d in range(DT):
                        nc.tensor.matmul(
                            ps[:], wt[d][:, 128 * ct:128 * (ct + 1)],
                            xTb_t[d][:], start=(d == 0), stop=(d == DT - 1))
                    if i < 4:
                        dest_t = qTb_p.tile([128, 512], BF16, name="qTb")
                        qTb_t.append(dest_t)
                        dest = dest_t[:]
                    else:
                        dest = kt_v[:, i - 4, tok]
                    if use_bqk:
                        nc.scalar.activation(dest, ps[:], AF.Identity,
                                             bias=bqk_sb[:, i:i + 1])
                    else:
                        nc.scalar.copy(dest, ps[:])

            wvt = []
            for d in range(DT):
                w = wv_p.tile([128, 512], BF16, name="w_v")
                nc.sync.dma_start(out=w[:], in_=wv[d])
                wvt.append(w)
            for tt in range(4):
                tg = 4 * c + tt
                ps = psM.tile([128, 512], F32, tag="mm", name="ps_v")
                for d in range(DT):
                    nc.tensor.matmul(
                        ps[:], xTb_t[d][:, 128 * tt:128 * (tt + 1)],
                        wvt[d][:], start=(d == 0),
                        stop=(d == DT - 1 and not use_bv))
                if use_bv:
                    nc.tensor.matmul(ps[:], ones_row_b[:], bv_sb[:],
                                     start=False, stop=True)
                v3 = v_v[:, tg, :].rearrange("p (h e) -> p h e", e=65)
                nc.scalar.copy(v3[:, :, 0:64],
                               ps[:].rearrange("p (h e) -> p h e", e=64))
                nc.vector.memset(v3[:, :, 64:65], 1.0)

            # ---- attention ----
            attnTb_t = [attnTb_p.tile([128, 512], BF16, tag="attnTb",
                                      name=f"attnTb{r}") for r in range(4)]
            den_t = den_p.tile([8, 512], F32, name="den")
            nt = 4 * (c + 1)
            for h in range(HPC):
                krt, koff = h // 2, 64 * (h % 2)
                q_ap = qTb_t[krt][koff:koff + 64, :]
                psa = psA.tile([65, 512], F32, name="psa")
                for t in range(nt):
                    j = t - 4 * c
                    qo = 128 * j if j >= 0 else 0
                    k_ap = kt_v[koff:koff + 64, krt, 128 * t:128 * (t + 1)]
                    ps_s = psS.tile([128, 512], F32, name="ps_s")
                    nc.tensor.matmul(ps_s[:, qo:], k_ap, q_ap[:, qo:],
                                     start=True, stop=True)
                    pt = pP.tile([128, 512], BF16, name="pt")
                    nc.scalar.activation(pt[:, qo:], ps_s[:, qo:], AF.Exp,
                                         bias=lna_sb[:, t:t + 1], scale=0.125)
                    if j >= 0:
                        nc.vector.tensor_mul(pt[:, qo:qo + 128],
                                             pt[:, qo:qo + 128], tri_sb[:])
                    nc.tensor.matmul(psa[:, qo:],
                                     v_v[:, t, 65 * h:65 * (h + 1)],
                                     pt[:, qo:], start=(t == 0),
                                     stop=(t == nt - 1))
                d1 = den1_p.tile([65, 512], F32, name="d1")
                nc.scalar.copy(d1[64:65, :], psa[64:65, :])
                nc.sync.dma_start(out=den_t[h:h + 1, :], in_=d1[64:65, :])
                if koff == 0:
                    nc.scalar.copy(attnTb_t[krt][0:64, :], psa[0:64, :])
                else:
                    t64 = tmp64_p.tile([64, 512], BF16, name="t64")
                    nc.scalar.copy(t64[:], psa[0:64, :])
                    nc.sync.dma_start(out=attnTb_t[krt][64:128, :],
                                      in_=t64[:])

            rec_t = recip_p.tile([8, 512], F32, name="rec")
            nc.vector.reciprocal(rec_t[:], den_t[:])
            rec_d = dram.tile([8, 512], F32, tag="recip_d", name="rec_d")
            nc.sync.dma_start(out=rec_d[:], in_=rec_t[:])
            for h in range(HPC):
                krt, koff = h // 2, 64 * (h % 2)
                rb = recipb_p.tile([128, 512], F32, name="rb")
                nc.sync.dma_start(
                    out=rb[:], in_=rec_d[h:h + 1, :].partition_broadcast(128))
                nc.vector.tensor_mul(attnTb_t[krt][koff:koff + 64, :],
                                     attnTb_t[krt][koff:koff + 64, :],
                                     rb[koff:koff + 64, :])

            # ---- attention projection + AR1 ----
            ar1_in = dram.tile([D, 512], F32, tag="ar1_in", name="ar1_in")
            ar1_out = dram.tile([D, 512], F32, tag="ar1_out", name="ar1_out")
            for cc in range(2):
                wpt = []
                for r in range(4):
                    w = wproj_p.tile([128, 512], BF16, name="w_pr")
                    nc.sync.dma_start(out=w[:], in_=wproj[4 * cc + r])
                    wpt.append(w)
                for ct in range(4):
                    dct = 4 * cc + ct
                    ps = psM.tile([128, 512], F32, tag="mm", name="ps_pr")
                    for r in range(4):
                        nc.tensor.matmul(
                            ps[:], wpt[r][:, 128 * ct:128 * (ct + 1)],
                            attnTb_t[r][:], start=(r == 0), stop=(r == 3))
                    ai = arin_p.tile([128, 512], F32, name="ai")
                    nc.scalar.copy(ai[:], ps[:])
                    nc.sync.dma_start(
                        out=ar1_in[:].rearrange("(i p) q -> p i q", p=128)
                        [:, dct, :], in_=ai[:])
            nc.gpsimd.collective_compute(
                "AllReduce", OP.add, replica_groups=PAIRS,
                ins=[ar1_in[:].opt()], outs=[ar1_out[:].opt()])
            state[("ar1", c)] = ar1_out

        # ============ stage 2: t1 + LN1 + fc + gelu + cproj + AR2 ===========
        def s2(c):
            ar1_out = state.pop(("ar1", c))
            art_t = []
            for i in range(DT):
                t1 = art_p.tile([128, 512], F32, name="t1")
                nc.sync.dma_start(
                    out=t1[:], in_=ar1_out[:]
                    .rearrange("(i p) q -> p i q", p=128)[:, i, :])
                xf2 = xf2_p.tile([128, 512], F32, name="xf2")
                nc.sync.dma_start(out=xf2[:], in_=xq[DT * c + i])
                nc.vector.tensor_add(t1[:], t1[:], xf2[:])
                if use_projb:
                    nc.vector.tensor_scalar_add(t1[:], t1[:],
                                                projb_sb[:, i:i + 1])
                art_t.append(t1)

            nTb_t = [None] * DT
            nT_d = dram.tile([DT, 128, 512], F32, tag="nT_d", name="nT_d")

            def mk_n(i, urstd_b, g_sb, b_sb, use_gb):
                nf = nf_p.tile([128, 512], F32, name="nf")
                nc.vector.tensor_sub(nf[:], art_t[i][:], urstd_b[:])
                if use_gb:
                    nc.vector.tensor_scalar(nf[:], nf[:], g_sb[:, i:i + 1],
                                            b_sb[:, i:i + 1], OP.mult, OP.add)
                nb = nTb_p.tile([128, 512], BF16, tag="nTb", name="nb")
                nc.vector.tensor_copy(nb[:], nf[:])
                nc.sync.dma_start(out=nT_d[i], in_=nf[:])
                nTb_t[i] = nb

            layernorm(art_t, mk_n, g1_sb, b1_sb, use_g1b1)
            state[("nT_d", c)] = nT_d

            # ---- fc + gelu ----
            gT_t = []
            for fg in range(4):
                wft = []
                for d in range(DT):
                    w = wfc_p.tile([128, 512], BF16, name="w_fc")
                    nc.sync.dma_start(out=w[:], in_=wfc[8 * fg + d])
                    wft.append(w)
                for fi in range(4):
                    f = 4 * fg + fi
                    ps = psM.tile([128, 512], F32, tag="mm", name="ps_fc")
                    for d in range(DT):
                        nc.tensor.matmul(
                            ps[:], wft[d][:, 128 * fi:128 * (fi + 1)],
                            nTb_t[d][:], start=(d == 0), stop=(d == DT - 1))
                    gt = gT_p.tile([128, 512], BF16, name="gt")
                    nc.scalar.activation(gt[:], ps[:], AF.Gelu_apprx_tanh,
                                         bias=fcb_sb[:, f:f + 1])
                    gT_t.append(gt)

            # ---- cproj + AR2 ----
            ar2_in = dram.tile([D, 512], F32, tag="ar2_in", name="ar2_in")
            ar2_out = dram.tile([D, 512], F32, tag="ar2_out", name="ar2_out")
            for p2 in range(4):
                wct = []
                for f in range(16):
                    w = wcproj_p.tile([128, 256], BF16, name="w_cp")
                    nc.sync.dma_start(out=w[:], in_=wcproj[16 * p2 + f])
                    wct.append(w)
                for ci in range(2):
                    dct = 2 * p2 + ci
                    ps = psM.tile([128, 512], F32, tag="mm", name="ps_cp")
                    for f in range(16):
                        nc.tensor.matmul(
                            ps[:], wct[f][:, 128 * ci:128 * (ci + 1)],
                            gT_t[f][:], start=(f == 0), stop=(f == 15))
                    mt = mT_p.tile([128, 512], F32, name="mt")
                    nc.scalar.copy(mt[:], ps[:])
                    nc.sync.dma_start(
                        out=ar2_in[:].rearrange("(i p) q -> p i q", p=128)
                        [:, dct, :], in_=mt[:])
            nc.gpsimd.collective_compute(
                "AllReduce", OP.add, replica_groups=PAIRS,
                ins=[ar2_in[:].opt()], outs=[ar2_out[:].opt()])
            state[("ar2", c)] = ar2_out

        # ================= stage 3: t2 + LN2 + store ========================
        def s3(c):
            ar2_out = state.pop(("ar2", c))
            nT_d = state.pop(("nT_d", c))
            mar_t = []
            for i in range(DT):
                m2 = mar_p.tile([128, 512], F32, name="m2")
                nc.sync.dma_start(
                    out=m2[:], in_=ar2_out[:]
                    .rearrange("(i p) q -> p i q", p=128)[:, i, :])
                nf2 = nf2_p.tile([128, 512], F32, name="nf2")
                nc.sync.dma_start(out=nf2[:], in_=nT_d[i])
                nc.vector.tensor_add(m2[:], m2[:], nf2[:])
                if use_cprojb:
                    nc.vector.tensor_scalar_add(m2[:], m2[:],
                                                cprojb_sb[:, i:i + 1])
                mar_t.append(m2)

            def mk_h(i, urstd_b, g_sb, b_sb, use_gb):
                ht = hT_p.tile([128, 512], F32, tag="hT", name="ht")
                nc.vector.tensor_sub(ht[:], mar_t[i][:], urstd_b[:])
                if use_gb:
                    nc.vector.tensor_scalar(ht[:], ht[:], g_sb[:, i:i + 1],
                                            b_sb[:, i:i + 1], OP.mult, OP.add)
                nc.sync.dma_start(out=out[DT * c + i], in_=ht[:])

            layernorm(mar_t, mk_h, g2_sb, b2_sb, use_g2b2)

        # pipelined emission, deep skew: ARs covered by ~2 stages of work
        for kind, c in [("1", 0), ("1", 1), ("1", 2), ("2", 0), ("1", 3),
                        ("2", 1), ("3", 0), ("2", 2), ("3", 1), ("2", 3),
                        ("3", 2), ("3", 3)]:
            {"1": s1, "2": s2, "3": s3}[kind](c)

    nc.compile()
    return nc


_cache = {}


def _get_program(flags):
    if flags not in _cache:
        _cache[flags] = _build(*flags)
    return _cache[flags]


def _pack(a, rows, cols):
    """[R, C] -> [R//rows * C//cols, rows, cols], row-tile-major."""
    R, C = a.shape
    return np.ascontiguousarray(
        a.reshape(R // rows, rows, C // cols, cols).transpose(0, 2, 1, 3)
        .reshape(-1, rows, cols))


def _prepare_inputs(inputs):
    x = np.asarray(inputs["x"], np.float32)
    weight = float(np.asarray(inputs["weight"]).reshape(-1)[0])
    linear_w = np.asarray(inputs["linear_w"], np.float32)
    linear_b = np.asarray(inputs["linear_b"], np.float32)
    proj_w = np.asarray(inputs["proj_w"], np.float32)
    proj_b = np.asarray(inputs["proj_b"], np.float32)
    ln1_g = np.asarray(inputs["ln1_g"], np.float32)
    ln1_b = np.asarray(inputs["ln1_b"], np.float32)
    fc_w = np.asarray(inputs["fc_w"], np.float32)
    fc_b = np.asarray(inputs["fc_b"], np.float32)
    cproj_w = np.asarray(inputs["cproj_w"], np.float32)
    cproj_b = np.asarray(inputs["cproj_b"], np.float32)
    ln2_g = np.asarray(inputs["ln2_g"], np.float32)
    ln2_b = np.asarray(inputs["ln2_b"], np.float32)
    idx = np.asarray(inputs["idx"]).astype(np.int64).reshape(-1)
    forcing = bool(np.asarray(inputs["attn_forcing"]).reshape(-1)[0])

    flags = (
        bool(linear_b[:2048].any()),      # use_bqk
        bool(linear_b[2048:].any()),      # use_bv
        bool(proj_b.any()),
        bool(cproj_b.any()),
        bool((ln1_g != 1).any() or ln1_b.any()),
        bool((ln2_g != 1).any() or ln2_b.any()),
    )

    if forcing:
        lnw = float(np.log(weight)) if weight >= 1e-37 else -1e9
        pos = np.arange(S)
        lna_all = np.where(pos[None, :] >= idx[:, None], lnw, 0.0) \
            .astype(np.float32)
    else:
        lna_all = np.zeros((B, S), np.float32)

    tri_np = np.triu(np.ones((128, 128), np.float32)).astype(BF)

    in_maps = []
    for core in range(N_CORES):
        b, r = core // 2, core % 2
        q_cols = slice(512 * r, 512 * (r + 1))
        k_cols = slice(1024 + 512 * r, 1024 + 512 * (r + 1))
        v_cols = slice(2048 + 512 * r, 2048 + 512 * (r + 1))
        xT = np.ascontiguousarray(x[b].T)                       # [D, S]
        wqk_full = np.concatenate([linear_w[:, q_cols], linear_w[:, k_cols]],
                                  axis=1)                       # [D, 1024]
        # _pack gives (row-tile, col-tile) order; kernel indexes are
        # (col-chunk, row-tile) for wqk/wproj/wfc, (col-pass, row-tile)
        # for wcproj, (chunk, row-tile) for xq -- so swap the axes.
        def swap(p, n_rt, n_ct):
            t = p.reshape(n_rt, n_ct, p.shape[1], p.shape[2])
            return np.ascontiguousarray(
                t.transpose(1, 0, 2, 3).reshape(-1, p.shape[1], p.shape[2]))

        in_maps.append({
            "xq": swap(_pack(xT, 128, 512), DT, NCH),            # (c, d)
            "wqk": swap(_pack(wqk_full.astype(BF), 128, 512), 8, 2),  # (cc,d)
            "bqk": np.ascontiguousarray(
                np.concatenate([linear_b[q_cols], linear_b[k_cols]])),
            "wv": _pack(linear_w[:, v_cols].astype(BF), 128, 512),    # (d)
            "bv": np.ascontiguousarray(linear_b[v_cols]).astype(BF),
            "wproj": swap(_pack(proj_w[512 * r:512 * (r + 1), :].astype(BF),
                                128, 512), 4, 2),                # (cc, r)
            "projb": proj_b,
            "wfc": swap(_pack(fc_w[:, 2048 * r:2048 * (r + 1)].astype(BF),
                              128, 512), 8, 4),                  # (fg, d)
            "fcb": np.ascontiguousarray(fc_b[2048 * r:2048 * (r + 1)]),
            "wcproj": swap(_pack(cproj_w[2048 * r:2048 * (r + 1), :]
                                 .astype(BF), 128, 256), 16, 4),  # (p2, f)
            "cprojb": cproj_b,
            "g1": ln1_g, "b1": ln1_b, "g2": ln2_g, "b2": ln2_b,
            "lna": lna_all[b],
            "tri": tri_np,
        })
    return flags, in_maps


def _unpack_out(o):
    """[NCH*DT, 128, 512] tiles (c, i) -> [S, D] token-major."""
    hT = o.reshape(NCH, DT, 128, 512).transpose(1, 2, 0, 3).reshape(D, S)
    return hT.T


def _run(inputs, trace=False):
    flags, in_maps = _prepare_inputs(inputs)
    nc = _get_program(flags)
    r = run_bass_kernel_spmd(nc, in_maps, core_ids=list(range(N_CORES)),
                             trace=trace)
    outs = np.stack(
        [np.ascontiguousarray(_unpack_out(r.results[2 * b]["out"]))
         for b in range(B)], axis=0).astype(np.float32)
    return outs, r


def kernel(**inputs) -> np.ndarray:
    outs, _ = _run(inputs, trace=False)
    return outs
